# revision 1
# baseline (speedup 1.0000x reference)
"""Trainium2 Bass kernel for LocalNodeAttentionHeadSum.

Computation (per batch b, pixel p=(h,w)):
    q[d,p]   = sum_c x[c,TMID,p] Wq[c,d] + bq[d]
    k[t,d]   = sum_c nodes[t,c] Wk[c,d] + bk[d]
    s[t,p]   = sum_d q[d,p] k[t,d];  alpha = softmax_t(s)
    y[d,p]   = sum_t alpha[t,p] * (sum_c x[c,t,p] Wv[c,d] + bv[d])
             = sum_c (sum_t alpha[t,p] x[c,t,p]) Wv[c,d] + bv[d]   (sum_t alpha = 1)
    out[c,p] = sum_d y[d,p] Wo[d,c] + bo[c]

Sharding: data-parallel over batch B=32 across 8 cores (4 batches/core).

Two algebraic reductions keep the kernel HBM-bound (~22.5 MB x shard/core):
  * the softmax-weighted temporal sum commutes with the value projection
    (linearity + sum_t alpha = 1), cutting value-projection PE work 7x;
  * the query projection folds into the scores: s = x_mid.T (Wq k.T) + bq.k,
    with Wqk = Wq @ k.T ([C, T]) precomputed once per kernel, so the
    per-batch score cost is 8 thin matmuls instead of a full [C,D] projection.

The kernel is emitted so the DMA stream never starves: all weight/constant
loads are front-loaded on the GPSIMD (SWDGE) queue while the x batches
stream on the SP (HWDGE) queue. Each fp32 x batch is transient - it is
downcast to bf16 (casts spread over GPSIMD and ACT) and its fp32 middle
frame extracted, then its staging tiles recycle. Softmax runs in the
transposed [T, pixels] layout (per-pixel max via GPSIMD partition
all-reduce, normalizer via ones-matmul, one exp per batch). The batch loop
is software-pipelined in emission order so no engine's in-order queue has a
later batch's stalled work ahead of an earlier batch's ready work.
"""

import sys

for _p in ("/opt/trn_rl_repo",):
    if _p not in sys.path:
        sys.path.insert(0, _p)

from contextlib import ExitStack

import numpy as np

import concourse.bass as bass
import concourse.tile as tile
from concourse import bacc, mybir, masks, bass_isa
from concourse.bass_utils import run_bass_kernel_spmd

F32 = mybir.dt.float32
BF16 = mybir.dt.bfloat16

# Problem shapes (hardcoded per contract)
B, C, T, H, W = 32, 1024, 7, 14, 14
D = 512
NCORES = 8
BL = B // NCORES          # 4 batches per core
HWF = H * W               # 196
THW = T * HWF             # 1372
CC = C // 128             # 8 chunks over input channels
HC = CC // 2              # chunks per half-batch staging tile
DC = D // 128             # 4 chunks over inter channels
TMID = T // 2             # 3 (middle frame)
SLOT = 256                # psum slot stride for alpha broadcast (bank-safe)

Exp = mybir.ActivationFunctionType.Exp
Identity = mybir.ActivationFunctionType.Identity


def build_program():
    nc = bacc.Bacc("TRN2", target_bir_lowering=False, debug=False)

    x_d = nc.dram_tensor("x_window", [BL, C, T, H, W], F32, kind="ExternalInput").ap()
    nodes_d = nc.dram_tensor("nodes", [T, D], F32, kind="ExternalInput").ap()
    wq_d = nc.dram_tensor("Wq", [C, D], F32, kind="ExternalInput").ap()
    bq_d = nc.dram_tensor("bq", [D], F32, kind="ExternalInput").ap()
    wk_d = nc.dram_tensor("Wk", [D, D], F32, kind="ExternalInput").ap()
    bk_d = nc.dram_tensor("bk", [D], F32, kind="ExternalInput").ap()
    wv_d = nc.dram_tensor("Wv", [C, D], F32, kind="ExternalInput").ap()
    bv_d = nc.dram_tensor("bv", [D], F32, kind="ExternalInput").ap()
    wo_d = nc.dram_tensor("Wo", [D, C], F32, kind="ExternalInput").ap()
    bo_d = nc.dram_tensor("bo", [C], F32, kind="ExternalInput").ap()
    out_d = nc.dram_tensor("out", [BL, C, 1, H, W], F32, kind="ExternalOutput").ap()

    x_r = x_d.rearrange("b (cc p) t h w -> b p cc (t h w)", p=128)
    out_r = out_d.rearrange("b (cc p) o h w -> b p cc (o h w)", p=128)
    wq_r = wq_d.rearrange("(cc p) d -> cc p d", p=128)
    wk_r = wk_d.rearrange("(dc p) d -> dc p d", p=128)
    wv_r = wv_d.rearrange("(cc p) d -> cc p d", p=128)
    wo_r = wo_d.rearrange("(dc p) (hc c) -> dc hc p c", p=128, hc=2)
    bq_r = bq_d.rearrange("(dc p) -> p dc", p=128)
    bk_r = bk_d.rearrange("(o d) -> o d", o=1)
    bv_r = bv_d.rearrange("(dc p) -> p dc", p=128)
    bo_r = bo_d.rearrange("(cc p) -> p cc", p=128)

    with tile.TileContext(nc) as tc, ExitStack() as ctx:
        cpool = ctx.enter_context(tc.tile_pool(name="const", bufs=1))
        wpool = ctx.enter_context(tc.tile_pool(name="wts", bufs=1))
        xpool = ctx.enter_context(tc.tile_pool(name="x", bufs=6))
        xbpool = ctx.enter_context(tc.tile_pool(name="xbf", bufs=4))
        xmpool = ctx.enter_context(tc.tile_pool(name="xmid", bufs=2))
        tpool = ctx.enter_context(tc.tile_pool(name="tmp", bufs=2))
        spool = ctx.enter_context(tc.tile_pool(name="sb", bufs=2))
        ypool = ctx.enter_context(tc.tile_pool(name="y", bufs=6))
        xwpool = ctx.enter_context(tc.tile_pool(name="xw", bufs=12))
        smpool = ctx.enter_context(tc.tile_pool(name="sm", bufs=2))
        obpool = ctx.enter_context(tc.tile_pool(name="ob", bufs=1))
        ps_mm = ctx.enter_context(tc.tile_pool(name="psmm", bufs=4, space="PSUM"))
        ps_ab = ctx.enter_context(tc.tile_pool(name="psab", bufs=1, space="PSUM"))

        # ---- pipeline stage definitions (weights referenced via closure) ----
        state = {}

        def stage_load(b):
            # fp32 middle-frame slices first: scores/softmax unblock after
            # ~0.8 MB instead of the full 5.6 MB window
            xmid = xmpool.tile([128, CC * HWF], F32, tag="xmid")
            nc.sync.dma_start(
                xmid[:].rearrange("p (cc f) -> p cc f", f=HWF),
                x_r[b][:, :, TMID * HWF : (TMID + 1) * HWF],
            )
            qs = []
            for q in range(4):
                xq = xpool.tile([128, 2 * THW], F32, tag="xf")
                nc.sync.dma_start(
                    xq[:].rearrange("p (cc f) -> p cc f", f=THW),
                    x_r[b, :, 2 * q : 2 * q + 2],
                )
                qs.append(xq)
            state[b] = {"xf": qs, "xmid": xmid}

        CAST_ACT = {0, 2}
        CAST_DVE = {1}

        def stage_cast(b):
            st = state[b]
            qs = st["xf"]
            xba = xbpool.tile([128, HC * THW], BF16, tag="xb")
            xbb = xbpool.tile([128, HC * THW], BF16, tag="xb")
            for q, xq in enumerate(qs):
                for i in range(2):
                    cc = 2 * q + i
                    xb = xba if cc < HC else xbb
                    dst = xb[:, (cc % HC) * THW : (cc % HC + 1) * THW]
                    src_ = xq[:, i * THW : (i + 1) * THW]
                    if cc in CAST_ACT:
                        nc.scalar.copy(dst, src_)
                    elif cc in CAST_DVE:
                        nc.vector.tensor_copy(dst, src_)
                    else:
                        nc.gpsimd.tensor_copy(dst, src_)
            st["xbf"] = (xba, xbb)
            del st["xf"]

        def stage_scores(b):
            st = state[b]
            xmid = st["xmid"]
            # transposed scores sT[t,p] directly from x mid slices via Wqk
            stp = ps_mm.tile([T, HWF], F32, tag="mm")
            for cc in range(CC):
                nc.tensor.matmul(
                    stp[:],
                    wqk_sb[cc][:],
                    xmid[:, cc * HWF : (cc + 1) * HWF],
                    start=(cc == 0),
                    stop=(cc == CC - 1),
                )
            s_sb = smpool.tile([T, HWF], F32, tag="s")
            nc.scalar.activation(s_sb[:], stp[:], Identity, bias=sb0[:], scale=1.0)
            # softmax over t (partition dim, T=7)
            mx = smpool.tile([T, HWF], F32, tag="mx")
            nc.gpsimd.partition_all_reduce(
                mx[:], s_sb[:], channels=T, reduce_op=bass_isa.ReduceOp.max
            )
            sm = smpool.tile([T, HWF], F32, tag="smx")
            nc.vector.tensor_sub(sm[:], s_sb[:], mx[:])
            e_sb = smpool.tile([T, HWF], F32, tag="e")
            nc.scalar.activation(e_sb[:], sm[:], Exp, bias=0.0, scale=1.0)
            zp = ps_mm.tile([1, HWF], F32, tag="mm")
            nc.tensor.matmul(zp[:], ones7[:], e_sb[:], start=True, stop=True)
            rz = smpool.tile([1, HWF], F32, tag="rz")
            nc.vector.reciprocal_approx_fast(rz[:], zp[:])
            rb = ps_mm.tile([T, HWF], F32, tag="mm")
            nc.tensor.matmul(rb[:], ones_f[0:1, 0:T], rz[:], start=True, stop=True)
            aT = smpool.tile([T, HWF], BF16, tag="aT")
            nc.vector.tensor_mul(aT[:], e_sb[:], rb[:])
            # broadcast alpha rows across 128 partitions via indicator matmuls
            abp = ps_ab.tile([128, T * SLOT], F32, tag="ab")
            for t in range(T):
                nc.tensor.matmul(
                    abp[:, t * SLOT : t * SLOT + HWF],
                    Es[t],
                    aT[:],
                    start=True,
                    stop=True,
                )
            ab = spool.tile([128, THW], BF16, tag="ab")
            nc.scalar.copy(
                ab[:].rearrange("p (t s) -> p t s", s=HWF),
                abp[:].rearrange("p (t s) -> p t s", s=SLOT)[:, :, 0:HWF],
            )
            st["ab"] = ab

        def stage_wsum(b):
            st = state[b]
            (xba, xbb), ab = st["xbf"], st["ab"]
            # weighted temporal sum: xw[c,p] = sum_t alpha[t,p] x[c,t,p] (bf16 DVE)
            xw = []
            for cc in range(CC):
                src = (xba if cc < HC else xbb)[
                    :, (cc % HC) * THW : (cc % HC + 1) * THW
                ]
                tm = tpool.tile([128, THW], BF16, tag="tm")
                nc.vector.tensor_mul(tm[:], src, ab[:])
                s1 = tpool.tile([128, 3 * HWF], BF16, tag="s1")
                nc.vector.tensor_add(
                    s1[:], tm[:, 0 : 3 * HWF], tm[:, 3 * HWF : 6 * HWF]
                )
                s2 = tpool.tile([128, HWF], BF16, tag="s2")
                nc.vector.tensor_add(s2[:], s1[:, 0:HWF], s1[:, HWF : 2 * HWF])
                s3 = tpool.tile([128, HWF], BF16, tag="s3")
                nc.vector.tensor_add(s3[:], s2[:], s1[:, 2 * HWF : 3 * HWF])
                xwt = xwpool.tile([128, HWF], BF16, tag="xw")
                nc.vector.tensor_add(xwt[:], s3[:], tm[:, 6 * HWF : 7 * HWF])
                xw.append(xwt)
            st["xw"] = xw

        def stage_proj(b):
            st = state[b]
            xw = st["xw"]
            # value projection on the weighted sum (bf16), bias via ACT
            y_sb = []
            for dd in range(DC):
                yp = ps_mm.tile([128, HWF], F32, tag="mm")
                for cc in range(CC):
                    nc.tensor.matmul(
                        yp[:],
                        wv_sb[cc][:, dd * 128 : (dd + 1) * 128],
                        xw[cc][:],
                        start=(cc == 0),
                        stop=(cc == CC - 1),
                    )
                yb = ypool.tile([128, HWF], BF16, tag="y")
                nc.scalar.activation(
                    yb[:], yp[:], Identity, bias=bvc[:, dd : dd + 1], scale=1.0
                )
                y_sb.append(yb)
            # output projection (bf16), bias via ACT, one merged DMA out
            ob = obpool.tile([128, CC * HWF], F32, tag="ob")
            for cc in range(CC):
                op = ps_mm.tile([128, HWF], F32, tag="mm")
                for dd in range(DC):
                    nc.tensor.matmul(
                        op[:],
                        wo_sb[dd][:, cc * 128 : (cc + 1) * 128],
                        y_sb[dd][:],
                        start=(dd == 0),
                        stop=(dd == DC - 1),
                    )
                nc.scalar.activation(
                    ob[:, cc * HWF : (cc + 1) * HWF],
                    op[:],
                    Identity,
                    bias=boc[:, cc : cc + 1],
                    scale=1.0,
                )
            nc.sync.dma_start(out_r[b], ob[:].rearrange("p (cc f) -> p cc f", f=HWF))
            del state[b]

        # ---- constants (merged DMAs, Pool/SWDGE queue) ----
        ones_f = cpool.tile([1, HWF], F32)
        nc.gpsimd.memset(ones_f[:], 1.0)
        ones7 = cpool.tile([T, 1], F32)
        nc.gpsimd.memset(ones7[:], 1.0)
        import ml_dtypes

        e_np = np.zeros((T, T * 128), dtype=ml_dtypes.bfloat16)
        for t in range(T):
            e_np[t, t * 128 : (t + 1) * 128] = 1.0
        e_dram = nc.inline_tensor(e_np, name="e_ind")
        e_all = cpool.tile([T, T * 128], BF16)
        nc.gpsimd.dma_start(e_all[:], e_dram.ap())
        Es = [e_all[:, t * 128 : (t + 1) * 128] for t in range(T)]

        bk_sb = cpool.tile([1, D], F32)
        nc.gpsimd.dma_start(bk_sb[:], bk_r[0])
        bqc = cpool.tile([128, DC], F32)
        nc.gpsimd.dma_start(bqc[:], bq_r)
        bvc = cpool.tile([128, DC], F32)
        nc.gpsimd.dma_start(bvc[:], bv_r)
        boc = cpool.tile([128, CC], F32)
        nc.gpsimd.dma_start(boc[:], bo_r)
        nodes_sb = cpool.tile([T, D], F32)
        nc.gpsimd.dma_start(nodes_sb[:], nodes_d[:, :])
        ident7 = cpool.tile([T, T], F32)
        masks.make_identity(nc, ident7[:])
        ident128 = cpool.tile([128, 128], F32)
        masks.make_identity(nc, ident128[:])

        # ---- weights + attention precomputes (DMAs on Pool; x owns SP) ----
        wv_sb = []
        wo_sb = []
        with (
            tc.tile_pool(name="stg", bufs=8) as stg,
            tc.tile_pool(name="stgw", bufs=1) as stgw,
        ):
            wk_sb = []
            for dd in range(DC):
                w = stg.tile([128, D], F32, tag="wstg")
                nc.gpsimd.dma_start(w[:], wk_r[dd])
                wk_sb.append(w)

            # nodesT + keys: kT[d_chunk] = (Wk.T @ nodes.T + bk) as [128, T]
            nodesT_sb = []
            for dd in range(DC):
                tp = ps_mm.tile([128, T], F32, tag="mm")
                nc.tensor.transpose(
                    tp[:], nodes_sb[:, dd * 128 : (dd + 1) * 128], ident7[:]
                )
                nt = cpool.tile([128, T], F32, tag=f"nT{dd}")
                nc.scalar.copy(nt[:], tp[:])
                nodesT_sb.append(nt)
            kT_sb = []
            for dd in range(DC):
                kp = ps_mm.tile([128, T], F32, tag="mm")
                for i in range(DC):
                    nc.tensor.matmul(
                        kp[:],
                        wk_sb[i][:, dd * 128 : (dd + 1) * 128],
                        nodesT_sb[i][:],
                        start=(i == 0),
                        stop=False,
                    )
                nc.tensor.matmul(
                    kp[:],
                    bk_sb[0:1, dd * 128 : (dd + 1) * 128],
                    ones_f[0:1, 0:T],
                    start=False,
                    stop=True,
                )
                kt = cpool.tile([128, T], F32, tag=f"kT{dd}")
                nc.scalar.copy(kt[:], kp[:])
                kT_sb.append(kt)

            # Wqk[c_chunk] = Wq @ k.T as [128, T] per c chunk (via WqT blocks)
            wqk_sb = []
            for cc in range(CC):
                wqrow = stg.tile([128, D], F32, tag="wstg")
                nc.gpsimd.dma_start(wqrow[:], wq_r[cc])
                wqts_sb = []
                for dd in range(DC):
                    wqt = ps_mm.tile([128, 128], F32, tag="mm")
                    nc.tensor.transpose(
                        wqt[:], wqrow[:, dd * 128 : (dd + 1) * 128], ident128[:]
                    )
                    wqts = stgw.tile([128, 128], F32, tag=f"wqts{dd}")
                    nc.scalar.copy(wqts[:], wqt[:])
                    wqts_sb.append(wqts)
                qkp = ps_mm.tile([128, T], F32, tag="mm")
                for dd in range(DC):
                    nc.tensor.matmul(
                        qkp[:],
                        wqts_sb[dd][:],
                        kT_sb[dd][:],
                        start=(dd == 0),
                        stop=(dd == DC - 1),
                    )
                wqk = cpool.tile([128, T], F32, tag=f"wqk{cc}")
                nc.scalar.copy(wqk[:], qkp[:])
                wqk_sb.append(wqk)

            # score bias sb0[t] = bq . k[t,:]  (per-partition bias in [T,p] layout)
            sbp = ps_mm.tile([T, 1], F32, tag="mm")
            for dd in range(DC):
                nc.tensor.matmul(
                    sbp[:],
                    kT_sb[dd][:],
                    bqc[:, dd : dd + 1],
                    start=(dd == 0),
                    stop=(dd == DC - 1),
                )
            sb0 = cpool.tile([T, 1], F32)
            nc.scalar.copy(sb0[:], sbp[:])

            # ---- software-pipelined emission ----
            stage_load(0)
            stage_cast(0)
            stage_load(1)
            stage_cast(1)


            stage_scores(0)
            stage_scores(1)
            # value/output weights -> bf16 (staged on SP between x1 and x2,
            # DVE casts); the DMA track stays dense either way.
            for cc in range(CC):
                s = stg.tile([128, D], F32, tag="wstg")
                nc.sync.dma_start(s[:], wv_r[cc])
                w = wpool.tile([128, D], BF16, tag=f"wv{cc}")
                nc.scalar.copy(w[:], s[:])
                wv_sb.append(w)
            for dd in range(DC):
                w = wpool.tile([128, C], BF16, tag=f"wo{dd}")
                for hc in range(2):
                    s = stg.tile([128, D], F32, tag="wstg")
                    nc.sync.dma_start(s[:], wo_r[dd, hc])
                    nc.vector.tensor_copy(w[:, hc * D : (hc + 1) * D], s[:])
                wo_sb.append(w)
            stage_wsum(0)
            stage_load(2)
            stage_scores(2)
            stage_cast(2)
            stage_proj(0)
            stage_wsum(1)
            stage_load(3)
            stage_scores(3)
            stage_cast(3)
            stage_proj(1)
            stage_wsum(2)
            stage_proj(2)
            stage_wsum(3)
            stage_proj(3)

    nc.compile()
    return nc


_PROG = None


def _get_prog():
    global _PROG
    if _PROG is None:
        _PROG = build_program()
    return _PROG


def _shard_inputs(inputs):
    f = lambda k: np.ascontiguousarray(np.asarray(inputs[k], dtype=np.float32))
    x = f("x_window")
    shared = {k: f(k) for k in ("nodes", "Wq", "bq", "Wk", "bk", "Wv", "bv", "Wo", "bo")}
    in_maps = []
    for i in range(NCORES):
        m = dict(shared)
        m["x_window"] = np.ascontiguousarray(x[i * BL : (i + 1) * BL])
        in_maps.append(m)
    return in_maps


def kernel(**inputs):
    nc = _get_prog()
    in_maps = _shard_inputs(inputs)
    res = run_bass_kernel_spmd(nc, in_maps, core_ids=list(range(NCORES)))
    return np.concatenate([res.results[i]["out"] for i in range(NCORES)], axis=0)



# revision 3
# speedup vs baseline: 1.5442x; 1.5442x over previous
"""Trainium2 Bass kernel for LocalNodeAttentionHeadSum.

Computation (per batch b, pixel p=(h,w)):
    q[d,p]   = sum_c x[c,TMID,p] Wq[c,d] + bq[d]
    k[t,d]   = sum_c nodes[t,c] Wk[c,d] + bk[d]
    s[t,p]   = sum_d q[d,p] k[t,d];  alpha = softmax_t(s)
    y[d,p]   = sum_t alpha[t,p] * (sum_c x[c,t,p] Wv[c,d] + bv[d])
             = sum_c (sum_t alpha[t,p] x[c,t,p]) Wv[c,d] + bv[d]   (sum_t alpha = 1)
    out[c,p] = sum_d y[d,p] Wo[d,c] + bo[c]

Sharding: data-parallel over batch B=32 across 8 cores (4 batches/core).

Weight pre-folding (host, numpy): the query projection and key path are
batch-independent, so  Wqk = Wq @ (nodes @ Wk + bk).T  ([C, T]) and
sb0[t] = bq . k[t]  are computed once on the host.  The device never sees
Wq/Wk/nodes/bq/bk; scores are 8 thin [128c x 7] matmuls per batch.
Wv / Wo ship from the host already cast to fp16.

DMA regime (the kernel is HBM-bound): x streams through fp32->fp16
casting SWDGE DMAs (gpsimd), which move half the bytes into SBUF that an
fp32 load would.  Each batch is two DMAs - the t=0..3 frames (includes
the middle frame, unlocking scores/softmax early) and the t=4..6 frames -
with every descriptor a contiguous >=1 KB run.  The output is stored as
fp16 with per-partition-contiguous 3 KB descriptors and widened to fp32
on the host.

Engine split per batch: PE does scores + alpha row-broadcast + the final
5-way temporal accumulation + both projections; DVE does the alpha
multiplies and the first add level; ACT applies biases, exp and
PSUM->SBUF moves; GPSIMD does DMA descriptor gen and the per-pixel
partition max for softmax.
"""

import sys

for _p in ("/opt/trn_rl_repo",):
    if _p not in sys.path:
        sys.path.insert(0, _p)

from contextlib import ExitStack

import numpy as np

import concourse.bass as bass
import concourse.tile as tile
from concourse import bacc, mybir, bass_isa
from concourse.bass_utils import run_bass_kernel_spmd

F32 = mybir.dt.float32
F16 = mybir.dt.float16

# Problem shapes (hardcoded per contract)
B, C, T, H, W = 32, 1024, 7, 14, 14
D = 512
NCORES = 8
BL = B // NCORES          # 4 batches per core
HWF = H * W               # 196
THW = T * HWF             # 1372
CC = C // 128             # 8 chunks over input channels
DC = D // 128             # 4 chunks over inter channels
TMID = T // 2             # 3 (middle frame)
TA = 4                    # first t-group (t=0..3, includes TMID)
TB = T - TA               # second t-group (t=4..6)
FA = TA * HWF             # 784
FB = TB * HWF             # 588
PSLOT = 512               # full psum bank (f32 elems) per small tile

Exp = mybir.ActivationFunctionType.Exp
Identity = mybir.ActivationFunctionType.Identity


def build_program():
    nc = bacc.Bacc("TRN2", target_bir_lowering=False, debug=False)

    x_d = nc.dram_tensor("x_window", [BL, C, T, H, W], F32, kind="ExternalInput").ap()
    wqk_d = nc.dram_tensor("wqk", [128, CC * T], F16, kind="ExternalInput").ap()
    sb0_d = nc.dram_tensor("sb0", [T, 1], F32, kind="ExternalInput").ap()
    wv_d = nc.dram_tensor("Wv", [C, D], F16, kind="ExternalInput").ap()
    wo_d = nc.dram_tensor("Wo", [D, C], F16, kind="ExternalInput").ap()
    bv_d = nc.dram_tensor("bvp", [128, DC], F32, kind="ExternalInput").ap()
    bo_d = nc.dram_tensor("bop", [128, CC], F32, kind="ExternalInput").ap()
    out_d = nc.dram_tensor("out", [BL, 128, CC * HWF], F16, kind="ExternalOutput").ap()

    x_r = x_d.rearrange("b (cc p) t h w -> b p cc (t h w)", p=128)
    wv_r = wv_d.rearrange("(cc p) d -> p cc d", p=128)
    wo_r = wo_d.rearrange("(dc p) c -> p dc c", p=128)

    with tile.TileContext(nc) as tc, ExitStack() as ctx:
        cpool = ctx.enter_context(tc.tile_pool(name="const", bufs=1))
        wpool = ctx.enter_context(tc.tile_pool(name="wts", bufs=1))
        xapool = ctx.enter_context(tc.tile_pool(name="xa", bufs=2))
        xbpool = ctx.enter_context(tc.tile_pool(name="xb", bufs=2))
        tmapool = ctx.enter_context(tc.tile_pool(name="tma", bufs=2))
        tmbpool = ctx.enter_context(tc.tile_pool(name="tmb", bufs=2))
        s1pool = ctx.enter_context(tc.tile_pool(name="s1", bufs=2))
        xwpool = ctx.enter_context(tc.tile_pool(name="xw", bufs=12))
        smpool = ctx.enter_context(tc.tile_pool(name="sm", bufs=2))
        abpool = ctx.enter_context(tc.tile_pool(name="ab", bufs=2))
        ypool = ctx.enter_context(tc.tile_pool(name="y", bufs=8))
        obpool = ctx.enter_context(tc.tile_pool(name="ob", bufs=2))
        # PSUM: [128, PSLOT] f32 tiles = exactly one 2 KB bank each.
        ps_chunk = ctx.enter_context(tc.tile_pool(name="psc", bufs=4, space="PSUM"))
        ps_small = ctx.enter_context(tc.tile_pool(name="pss", bufs=2, space="PSUM"))
        ps_proj = ctx.enter_context(tc.tile_pool(name="psp", bufs=2, space="PSUM"))

        # ---- constants (tiny DMAs on the Pool/SWDGE queue) ----
        ones7 = cpool.tile([T, 1], F32)
        nc.gpsimd.memset(ones7[:], 1.0)
        ones1 = cpool.tile([1, T], F32)
        nc.gpsimd.memset(ones1[:], 1.0)

        e_np = np.zeros((T, T * 128), dtype=np.float16)
        for t in range(T):
            e_np[t, t * 128 : (t + 1) * 128] = 1.0
        e_dram = nc.inline_tensor(e_np, name="e_ind")
        e_all = cpool.tile([T, T * 128], F16)
        nc.gpsimd.dma_start(e_all[:], e_dram.ap())
        Es = [e_all[:, t * 128 : (t + 1) * 128] for t in range(T)]

        wqk_sb = cpool.tile([128, CC * T], F16)
        nc.gpsimd.dma_start(wqk_sb[:], wqk_d)
        sb0 = cpool.tile([T, 1], F32)
        nc.gpsimd.dma_start(sb0[:], sb0_d)
        bvc = cpool.tile([128, DC], F32)
        nc.gpsimd.dma_start(bvc[:], bv_d)
        boc = cpool.tile([128, CC], F32)
        nc.gpsimd.dma_start(boc[:], bo_d)

        state = {}

        # ---- pipeline stages ----
        def load_a(b):
            xa = xapool.tile([128, CC * FA], F16, tag="xa")
            nc.gpsimd.dma_start(
                xa[:].rearrange("p (cc f) -> p cc f", f=FA),
                x_r[b][:, :, 0:FA],
            )
            state[b] = {"xa": xa}

        def load_b(b):
            xb = xbpool.tile([128, CC * FB], F16, tag="xb")
            nc.gpsimd.dma_start(
                xb[:].rearrange("p (cc f) -> p cc f", f=FB),
                x_r[b][:, :, FA:THW],
            )
            state[b]["xb"] = xb

        def load_weights():
            wv_sb = wpool.tile([128, CC * D], F16)
            nc.sync.dma_start(
                wv_sb[:].rearrange("p (cc d) -> p cc d", d=D), wv_r
            )
            wo_sb = wpool.tile([128, DC * C], F16)
            nc.sync.dma_start(
                wo_sb[:].rearrange("p (dc c) -> p dc c", c=C), wo_r
            )
            return wv_sb, wo_sb

        def scores(b):
            st = state[b]
            xa = st["xa"]
            # scores sT[t,p] from the middle frame (inside the t=0..3 group)
            stp = ps_small.tile([T, PSLOT], F32, tag="pss")
            for cc in range(CC):
                nc.tensor.matmul(
                    stp[:, 0:HWF],
                    wqk_sb[:, cc * T : (cc + 1) * T],
                    xa[:, cc * FA + TMID * HWF : cc * FA + (TMID + 1) * HWF],
                    start=(cc == 0),
                    stop=(cc == CC - 1),
                )
            s_sb = smpool.tile([T, HWF], F32, tag="s")
            nc.scalar.activation(s_sb[:], stp[:, 0:HWF], Identity, bias=sb0[:], scale=1.0)
            # softmax over t (partition dim, T=7)
            mx = smpool.tile([T, HWF], F32, tag="mx")
            nc.gpsimd.partition_all_reduce(
                mx[:], s_sb[:], channels=T, reduce_op=bass_isa.ReduceOp.max
            )
            sm = smpool.tile([T, HWF], F32, tag="smx")
            nc.vector.tensor_sub(sm[:], s_sb[:], mx[:])
            e_sb = smpool.tile([T, HWF], F32, tag="e")
            nc.scalar.activation(e_sb[:], sm[:], Exp, bias=0.0, scale=1.0)
            zp = ps_small.tile([1, PSLOT], F32, tag="pss")
            nc.tensor.matmul(zp[:, 0:HWF], ones7[:], e_sb[:], start=True, stop=True)
            rz = smpool.tile([1, HWF], F32, tag="rz")
            nc.vector.reciprocal_approx_fast(rz[:], zp[:, 0:HWF])
            rb = ps_small.tile([T, PSLOT], F32, tag="pss")
            nc.tensor.matmul(rb[:, 0:HWF], ones1[:], rz[:], start=True, stop=True)
            aT = smpool.tile([T, HWF], F16, tag="aT")
            nc.vector.tensor_mul(aT[:], e_sb[:], rb[:, 0:HWF])
            # broadcast alpha rows across the 128 c-partitions (indicator matmuls)
            ab = abpool.tile([128, THW], F16, tag="ab")
            for t in range(T):
                abp = ps_chunk.tile([128, PSLOT], F32, tag="psc")
                nc.tensor.matmul(
                    abp[:, 0:HWF], Es[t], aT[:], start=True, stop=True
                )
                nc.scalar.copy(ab[:, t * HWF : (t + 1) * HWF], abp[:, 0:HWF])
            st["ab"] = ab

        def wsum_a(b):
            st = state[b]
            xa, ab = st["xa"], st["ab"]
            tma = tmapool.tile([128, CC * FA], F16, tag="tma")
            for cc in range(CC):
                nc.vector.tensor_mul(
                    tma[:, cc * FA : (cc + 1) * FA],
                    xa[:, cc * FA : (cc + 1) * FA],
                    ab[:, 0:FA],
                )
            # first add level: (t0+t2, t1+t3) per chunk, all chunks in one op
            s1a = s1pool.tile([128, CC * 2 * HWF], F16, tag="s1a")
            tav = tma[:].rearrange("p (cc f) -> p cc f", f=FA)
            nc.vector.tensor_add(
                s1a[:].rearrange("p (cc f) -> p cc f", f=2 * HWF),
                tav[:, :, 0 : 2 * HWF],
                tav[:, :, 2 * HWF : 4 * HWF],
            )
            st["s1a"] = s1a

        def wsum_b(b):
            st = state[b]
            xb, ab, s1a = st["xb"], st["ab"], st["s1a"]
            tmb = tmbpool.tile([128, CC * FB], F16, tag="tmb")
            for cc in range(CC):
                nc.vector.tensor_mul(
                    tmb[:, cc * FB : (cc + 1) * FB],
                    xb[:, cc * FB : (cc + 1) * FB],
                    ab[:, FA:THW],
                )
            # final 5-way accumulation on PE (identity-matmul accumulate):
            # xw = s1a[0:196] + s1a[196:392] + tmb[t4] + tmb[t5] + tmb[t6]
            xw = xwpool.tile([128, CC * HWF], F16, tag="xw")
            for cc in range(CC):
                xp = ps_chunk.tile([128, PSLOT], F32, tag="psc")
                movings = [
                    s1a[:, cc * 2 * HWF : cc * 2 * HWF + HWF],
                    s1a[:, cc * 2 * HWF + HWF : cc * 2 * HWF + 2 * HWF],
                    tmb[:, cc * FB : cc * FB + HWF],
                    tmb[:, cc * FB + HWF : cc * FB + 2 * HWF],
                    tmb[:, cc * FB + 2 * HWF : cc * FB + 3 * HWF],
                ]
                for i, mv in enumerate(movings):
                    nc.tensor.matmul(
                        xp[:, 0:HWF],
                        ident_bf[:],
                        mv,
                        start=(i == 0),
                        stop=(i == len(movings) - 1),
                    )
                nc.scalar.copy(xw[:, cc * HWF : (cc + 1) * HWF], xp[:, 0:HWF])
            st["xw"] = xw

        def proj(b):
            st = state[b]
            xw = st["xw"]
            # value projection (contract over c), bias via ACT
            y_sb = []
            for dd in range(DC):
                yp = ps_proj.tile([128, PSLOT], F32, tag="psp")
                for cc in range(CC):
                    nc.tensor.matmul(
                        yp[:, 0:HWF],
                        wv_sb[:, cc * D + dd * 128 : cc * D + (dd + 1) * 128],
                        xw[:, cc * HWF : (cc + 1) * HWF],
                        start=(cc == 0),
                        stop=(cc == CC - 1),
                    )
                yb = ypool.tile([128, HWF], F16, tag="y")
                nc.scalar.activation(
                    yb[:], yp[:, 0:HWF], Identity, bias=bvc[:, dd : dd + 1], scale=1.0
                )
                y_sb.append(yb)
            # output projection (contract over d), bias via ACT, merged store
            ob = obpool.tile([128, CC * HWF], F16, tag="ob")
            for cc in range(CC):
                op = ps_proj.tile([128, PSLOT], F32, tag="psp")
                for dd in range(DC):
                    nc.tensor.matmul(
                        op[:, 0:HWF],
                        wo_sb[:, dd * C + cc * 128 : dd * C + (cc + 1) * 128],
                        y_sb[dd][:],
                        start=(dd == 0),
                        stop=(dd == DC - 1),
                    )
                nc.scalar.activation(
                    ob[:, cc * HWF : (cc + 1) * HWF],
                    op[:, 0:HWF],
                    Identity,
                    bias=boc[:, cc : cc + 1],
                    scale=1.0,
                )
            nc.sync.dma_start(out_d[b], ob[:])
            del state[b]

        # identity for the PE accumulation adds (fp16 keeps 1 cycle/row)
        id_np = np.eye(128, dtype=np.float16)
        id_dram = nc.inline_tensor(id_np, name="id128")
        ident_bf = cpool.tile([128, 128], F16)
        nc.gpsimd.dma_start(ident_bf[:], id_dram.ap())

        # ---- software-pipelined emission ----
        load_a(0)
        load_b(0)
        wv_sb, wo_sb = load_weights()
        scores(0)
        load_a(1)
        load_b(1)
        wsum_a(0)
        wsum_b(0)
        scores(1)
        load_a(2)
        load_b(2)
        proj(0)
        wsum_a(1)
        wsum_b(1)
        scores(2)
        load_a(3)
        load_b(3)
        proj(1)
        wsum_a(2)
        wsum_b(2)
        scores(3)
        proj(2)
        wsum_a(3)
        wsum_b(3)
        proj(3)

    nc.compile()
    return nc


_PROG = None


def _get_prog():
    global _PROG
    if _PROG is None:
        _PROG = build_program()
    return _PROG


def _prep_inputs(inputs):
    f = lambda k: np.asarray(inputs[k], dtype=np.float32)
    x = np.ascontiguousarray(f("x_window"))
    nodes, Wq, bq, Wk, bk = f("nodes"), f("Wq"), f("bq"), f("Wk"), f("bk")
    Wv, bv, Wo, bo = f("Wv"), f("bv"), f("Wo"), f("bo")

    # host-side weight folding (batch-independent)
    k = nodes @ Wk + bk                       # [T, D]
    wqk = Wq @ k.T                            # [C, T]
    sb0 = (bq @ k.T).reshape(T, 1).astype(np.float32)          # [T, 1]
    wqk_p = np.ascontiguousarray(
        wqk.reshape(CC, 128, T).transpose(1, 0, 2).reshape(128, CC * T)
    ).astype(np.float16)
    bvp = np.ascontiguousarray(bv.reshape(DC, 128).T)          # [128, DC]
    bop = np.ascontiguousarray(bo.reshape(CC, 128).T)          # [128, CC]

    shared = {
        "wqk": wqk_p,
        "sb0": sb0,
        "Wv": Wv.astype(np.float16),
        "Wo": Wo.astype(np.float16),
        "bvp": bvp,
        "bop": bop,
    }
    in_maps = []
    for i in range(NCORES):
        m = dict(shared)
        m["x_window"] = np.ascontiguousarray(x[i * BL : (i + 1) * BL])
        in_maps.append(m)
    return in_maps


def _unshard_out(res):
    parts = []
    for i in range(NCORES):
        ob = np.asarray(res.results[i]["out"], dtype=np.float32)  # [BL,128,CC*HWF]
        ob = ob.reshape(BL, 128, CC, HWF).transpose(0, 2, 1, 3)   # [BL,CC,128,HWF]
        parts.append(ob.reshape(BL, C, 1, H, W))
    return np.concatenate(parts, axis=0)


def kernel(**inputs):
    nc = _get_prog()
    in_maps = _prep_inputs(inputs)
    res = run_bass_kernel_spmd(nc, in_maps, core_ids=list(range(NCORES)))
    return _unshard_out(res)
